# revision 5
# baseline (speedup 1.0000x reference)
"""Trainium2 Bass kernel for nn_CustomRNNmodel: embed -> 2-layer LSTM -> LN -> tied LM head.

Strategy (8 NeuronCores, SPMD, no collectives):
  - LSTM recurrence replicated on every core (per-step collectives are too
    expensive); vocab-dim of the tied LM head sharded 8 ways per core.
  - Two passes over time (all of layer 0, then layer 1) so each layer's
    input projection is a big parallel GEMM (X = x @ W_ih^T + b), leaving
    only W_hh @ h in the sequential loop.
  - All big GEMMs (input projections, LM head, embed transpose) run in
    bf16 (1 cycle/row on PE vs 4 for fp32); the recurrence stays fp32.
  - Recurrence matmuls: weights stationary, 4-way column tiling (M=32
    tiles at tile_position (0,32j)), K=128 chunks. Gate rows are permuted
    host-side so gates land in PSUM as [128 partitions, 4*slot] tiles with
    f/i/g/o contiguous col-blocks; h/c state tiles [128, 4*k] feed the next
    step's rhs directly (no transposes anywhere in the loop).

Layout bookkeeping (the invariant everything relies on):
  token index n = B*t + b  (b minor)
  gate row rho (after host permutation) = 32*c + m, c = 4*s + j,
    s = 8*blk + sg, blk in [f,i,g,o], p = 32*j + m, hidden unit u = 128*sg + p
  - PSUM gate tile for block blk: [p, 4*sg + b]
  - h/c state tile: [p, 4*sg + b]  <->  unit u = 128*sg + p  (sg = kappa)
  - next-step rhs for contraction chunk k = h[:, 4k:4k+4]
  - X (input projection) DRAM: [32 (s), 128 (p), N]
  - HT (hidden states) DRAM: [1024 (u), N]  (bf16)
"""
import numpy as np
import ml_dtypes
from contextlib import ExitStack

import concourse.bass as bass
import concourse.tile as tile
from concourse import bacc, mybir
from concourse.bass_utils import run_bass_kernel_spmd

V, H, L, B, NC = 32000, 1024, 2, 4, 8
VSH = V // NC                      # 4000 vocab rows per core
FP32 = mybir.dt.float32
BF16 = mybir.dt.bfloat16
NP_BF16 = ml_dtypes.bfloat16
EPS = 1e-5

# gate block order in our layout; reference order is [i, f, g, o]
BLK_TO_ORIG = [1, 0, 2, 3]         # blk 0=f 1=i 2=g 3=o -> orig gate index


def gate_row_perm():
    """perm[rho] = original row index in [0,4H) for permuted row rho."""
    rho = np.arange(4 * H)
    m = rho & 31
    c = rho >> 5
    j = c & 3
    s = c >> 2
    blk = s >> 3
    sg = s & 7
    u = 128 * sg + 32 * j + m
    orig_gate = np.array(BLK_TO_ORIG)[blk]
    return orig_gate * H + u


def _prep_w(w, np_dtype=np.float32):
    """[4H, H] weight -> permuted-transposed [H, 4H] contiguous."""
    perm = gate_row_perm()
    return np.ascontiguousarray(np.asarray(w)[perm, :].T.astype(np_dtype))


def _prep_b(b_ih, b_hh):
    perm = gate_row_perm()
    return np.ascontiguousarray(
        (np.asarray(b_ih) + np.asarray(b_hh))[perm].astype(np.float32))


def build_nc(T=1024, unroll=2, debug_outputs=False, staggered=True):
    """Build the full single-core SPMD program (same on all 8 cores; only the
    wvT input differs per core)."""
    nc = bacc.Bacc("TRN2", target_bir_lowering=False, debug=False)
    N = B * T                       # tokens
    NSL = min(512, N)               # n-slice width for GEMMs
    NB = N // NSL                   # n-slices
    KT = H // 128                   # 8 contraction chunks
    Sig = mybir.ActivationFunctionType.Sigmoid
    Tanh = mybir.ActivationFunctionType.Tanh
    Ident = mybir.ActivationFunctionType.Identity

    # ---- inputs ----
    ids = nc.dram_tensor("ids", [N], mybir.dt.int32, kind="ExternalInput")
    pidx = nc.dram_tensor("pidx", [N], mybir.dt.int32, kind="ExternalInput")
    wemb = nc.dram_tensor("wemb", [V, H], FP32, kind="ExternalInput")
    pos = nc.dram_tensor("pos", [T, H], FP32, kind="ExternalInput")
    wih0 = nc.dram_tensor("wih0", [H, 4 * H], BF16, kind="ExternalInput")
    whh0 = nc.dram_tensor("whh0", [H, 4 * H], FP32, kind="ExternalInput")
    wih1 = nc.dram_tensor("wih1", [H, 4 * H], BF16, kind="ExternalInput")
    whh1 = nc.dram_tensor("whh1", [H, 4 * H], FP32, kind="ExternalInput")
    bias0 = nc.dram_tensor("bias0", [4 * H], FP32, kind="ExternalInput")
    bias1 = nc.dram_tensor("bias1", [4 * H], FP32, kind="ExternalInput")
    k12 = nc.dram_tensor("k12", [2, VSH], BF16, kind="ExternalInput")
    wvT = nc.dram_tensor("wvT", [H, VSH], BF16, kind="ExternalInput")
    ident = nc.dram_tensor("ident", [128, 128], BF16, kind="ExternalInput")

    # ---- outputs / scratch DRAM ----
    logits = nc.dram_tensor("logits", [N, VSH], FP32, kind="ExternalOutput")
    kind_dbg = "ExternalOutput" if debug_outputs else "Internal"
    ftT = nc.dram_tensor("ftT", [H, N], BF16, kind=kind_dbg)
    x0d = nc.dram_tensor("x0d", [32, 128, N], FP32, kind=kind_dbg)
    h0T = nc.dram_tensor("h0T", [H, N], BF16, kind=kind_dbg)
    x1d = nc.dram_tensor("x1d", [32, 128, N], FP32, kind=kind_dbg)
    h1T = nc.dram_tensor("h1T", [H, N], BF16, kind=kind_dbg)
    statsd = nc.dram_tensor("statsd", [3, N], FP32, kind=kind_dbg)

    with tile.TileContext(nc) as tc:
        # ================= embed + transpose =================
        with tc.tile_pool(name="emb", bufs=3) as ep, \
             tc.tile_pool(name="emb_ps", bufs=4, space="PSUM") as epp, \
             tc.tile_pool(name="emb_c", bufs=1) as ecp:
            ident_sb = ecp.tile([128, 128], BF16, tag="ident")
            nc.sync.dma_start(ident_sb[:], ident[:, :])
            for q in range(N // 128):
                idt = ep.tile([128, 1], mybir.dt.int32, tag="idt")
                nc.sync.dma_start(idt[:], ids[128 * q:128 * (q + 1)].rearrange("(p o) -> p o", o=1))
                pidt = ep.tile([128, 1], mybir.dt.int32, tag="pidt")
                nc.sync.dma_start(pidt[:], pidx[128 * q:128 * (q + 1)].rearrange("(p o) -> p o", o=1))
                tok = ep.tile([128, H], FP32, tag="tok")
                nc.gpsimd.indirect_dma_start(
                    out=tok[:], out_offset=None, in_=wemb[:, :],
                    in_offset=bass.IndirectOffsetOnAxis(ap=idt[:, :1], axis=0))
                pst = ep.tile([128, H], FP32, tag="pst")
                nc.gpsimd.indirect_dma_start(
                    out=pst[:], out_offset=None, in_=pos[:, :],
                    in_offset=bass.IndirectOffsetOnAxis(ap=pidt[:, :1], axis=0))
                ftb = ep.tile([128, H], BF16, tag="ftb")
                nc.vector.tensor_add(ftb[:], tok[:], pst[:])
                for k in range(KT):
                    tp = epp.tile([128, 128], BF16)
                    nc.tensor.transpose(tp[:], ftb[:, 128 * k:128 * (k + 1)], ident_sb[:])
                    ftc = ep.tile([128, 128], BF16, tag="ftc")
                    nc.any.tensor_copy(ftc[:], tp[:])
                    nc.sync.dma_start(
                        ftT[128 * k:128 * (k + 1), 128 * q:128 * (q + 1)], ftc[:])

        # ============ helper: input-projection GEMM (bf16) ============
        # X = srcT-chunks^T @ Wprep + bias  -> xout [32, 128, N] fp32
        def xgemm(wdram, srcT, xout, biasdram):
            with tc.tile_pool(name="xg_w", bufs=1) as wp, \
                 tc.tile_pool(name="xg_sm", bufs=1) as smp, \
                 tc.tile_pool(name="xg_rhs", bufs=2) as rp, \
                 tc.tile_pool(name="xg_out", bufs=4) as op, \
                 tc.tile_pool(name="xg_ps", bufs=4, space="PSUM") as pp:
                wsb = wp.tile([128, KT * 4 * H], BF16, tag="wsb")
                for k in range(KT):
                    nc.sync.dma_start(
                        wsb[:, k * 4 * H:(k + 1) * 4 * H],
                        wdram[128 * k:128 * (k + 1), :])
                biassb = smp.tile([128, 32], FP32, tag="bias")
                nc.sync.dma_start(biassb[:], biasdram.rearrange("(s p) -> p s", p=128))
                for nb in range(NB):
                    n0 = nb * NSL
                    rhs = rp.tile([128, KT * NSL], BF16, tag="rhs")
                    for k in range(KT):
                        nc.sync.dma_start(
                            rhs[:, k * NSL:(k + 1) * NSL],
                            srcT[128 * k:128 * (k + 1), n0:n0 + NSL])
                    for s in range(32):
                        ps = pp.tile([128, NSL], FP32)
                        for k in range(KT):
                            nc.tensor.matmul(
                                ps[:],
                                wsb[:, 4 * H * k + 128 * s:4 * H * k + 128 * (s + 1)],
                                rhs[:, k * NSL:(k + 1) * NSL],
                                start=(k == 0), stop=(k == KT - 1))
                        ot = op.tile([128, NSL], FP32, tag="xo")
                        nc.scalar.activation(ot[:], ps[:], Ident,
                                             bias=biassb[:, s:s + 1], scale=1.0)
                        nc.sync.dma_start(xout[s, :, n0:n0 + NSL], ot[:])

        # ============ helper: LSTM recurrence pass (fp32) ============
        def lstm_pass(wdram, xd, hTout):
            hTr = hTout.rearrange("(sg p) n -> p sg n", p=128)
            with tc.tile_pool(name="rec_w", bufs=1) as wp, \
                 tc.tile_pool(name="rec_st", bufs=1) as sp, \
                 tc.tile_pool(name="rec_sc", bufs=2 * unroll) as scp, \
                 tc.tile_pool(name="rec_ps", bufs=unroll, space="PSUM") as rpp:
                wsb = wp.tile([128, KT * 4 * H], FP32, tag="wsb")
                for k in range(KT):
                    nc.sync.dma_start(
                        wsb[:, k * 4 * H:(k + 1) * 4 * H],
                        wdram[128 * k:128 * (k + 1), :])
                h = sp.tile([128, 32], FP32, tag="h")
                c = sp.tile([128, 32], FP32, tag="c")
                nc.vector.memset(h[:], 0.0)
                nc.vector.memset(c[:], 0.0)
                assert T % unroll == 0
                with tc.For_i(0, B * T, B * unroll,
                              staggered_reset=staggered,
                              hint_engines=(mybir.EngineType.PE,
                                            mybir.EngineType.DVE,
                                            mybir.EngineType.Activation,
                                            mybir.EngineType.SP)) as i:
                    for uu in range(unroll):
                        col = bass.ds(i + B * uu, B)
                        xt = scp.tile([128, 128], FP32, tag="xt")
                        nc.sync.dma_start(
                            xt[:].rearrange("p (s b) -> p s b", b=4),
                            xd[:, :, col].rearrange("s p b -> p s b"))
                        psb = [rpp.tile([128, 32], FP32, tag=f"ps{bb}", name=f"psb{bb}")
                               for bb in range(4)]
                        # gate blocks in order f, i, g, o
                        for blk in range(4):
                            for sg in range(8):
                                s = 8 * blk + sg
                                for k in range(KT):
                                    for j in range(4):
                                        cidx = 4 * s + j
                                        nc.tensor.matmul(
                                            psb[blk][32 * j:32 * (j + 1), 4 * sg:4 * (sg + 1)],
                                            wsb[:, 4 * H * k + 32 * cidx:4 * H * k + 32 * (cidx + 1)],
                                            h[:, 4 * k:4 * (k + 1)],
                                            start=(k == 0), stop=(k == KT - 1),
                                            tile_position=(0, 32 * j))
                        pre = [scp.tile([128, 32], FP32, tag=f"pre{bb}", name=f"pre{bb}")
                               for bb in range(4)]
                        for blk in range(4):
                            nc.vector.tensor_add(pre[blk][:], psb[blk][:],
                                                 xt[:, 32 * blk:32 * (blk + 1)])
                        F = scp.tile([128, 32], FP32, tag="F")
                        I = scp.tile([128, 32], FP32, tag="I")
                        G = scp.tile([128, 32], FP32, tag="G")
                        O = scp.tile([128, 32], FP32, tag="O")
                        nc.scalar.activation(F[:], pre[0][:], Sig)
                        t1 = scp.tile([128, 32], FP32, tag="t1")
                        nc.vector.tensor_mul(t1[:], F[:], c[:])
                        nc.scalar.activation(I[:], pre[1][:], Sig)
                        nc.scalar.activation(G[:], pre[2][:], Tanh)
                        t2 = scp.tile([128, 32], FP32, tag="t2")
                        nc.vector.tensor_mul(t2[:], I[:], G[:])
                        nc.vector.tensor_add(c[:], t1[:], t2[:])
                        TC = scp.tile([128, 32], FP32, tag="TC")
                        nc.scalar.activation(TC[:], c[:], Tanh)
                        nc.scalar.activation(O[:], pre[3][:], Sig)
                        nc.vector.tensor_mul(h[:], O[:], TC[:])
                        hb = scp.tile([128, 32], BF16, tag="hb")
                        nc.vector.tensor_copy(hb[:], h[:])
                        nc.sync.dma_start(
                            hTr[:, :, col],
                            hb[:].rearrange("p (sg b) -> p sg b", b=4))

        with nc.named_scope("xg0"):
            xgemm(wih0, ftT, x0d, bias0)
        with nc.named_scope("rec0"):
            lstm_pass(whh0, x0d, h0T)
        with nc.named_scope("xg1"):
            xgemm(wih1, h0T, x1d, bias1)
        with nc.named_scope("rec1"):
            lstm_pass(whh1, x1d, h1T)

        # ========== layernorm folded into LM head (bf16) ==========
        # logits[n,v] = r[n] * ( sum_u x[u,n]*Wv'[u,v] + (-mu[n])*K1[v] + sd[n]*K2[v] )
        # with Wv' = lnw*WvT (host), K1 = Wv@lnw, K2 = Wv@lnb (host), r = 1/sd.
        with tc.tile_pool(name="ln_res", bufs=1) as lp:
            h1sb = [lp.tile([128, N], BF16, tag=f"h1_{k}", name=f"h1sb{k}") for k in range(KT)]
            for k in range(KT):
                nc.sync.dma_start(h1sb[k][:], h1T[128 * k:128 * (k + 1), :])
            with tc.tile_pool(name="ln_tmp", bufs=1) as ltp, \
                 tc.tile_pool(name="ln_sc", bufs=2) as lsp, \
                 tc.tile_pool(name="ln_ps", bufs=2, space="PSUM") as lpp:
                ones = ltp.tile([128, 1], BF16, tag="ones")
                nc.vector.memset(ones[:], 1.0)
                mu = ltp.tile([1, N], FP32, tag="mu")
                msq = ltp.tile([1, N], FP32, tag="msq")
                for nb in range(NB):
                    n0 = nb * NSL
                    ps = lpp.tile([1, NSL], FP32, tag="lnps")
                    for k in range(KT):
                        nc.tensor.matmul(ps[:], ones[:], h1sb[k][:, n0:n0 + NSL],
                                         start=(k == 0), stop=(k == KT - 1))
                    # mu slot first holds -mean = -colsum/H
                    nc.vector.tensor_scalar_mul(mu[:, n0:n0 + NSL], ps[:], -1.0 / H)
                    sq = lsp.tile([128, NSL], BF16, tag="sq")
                    ps2 = lpp.tile([1, NSL], FP32, tag="lnps2")
                    for k in range(KT):
                        nc.vector.tensor_mul(sq[:], h1sb[k][:, n0:n0 + NSL],
                                             h1sb[k][:, n0:n0 + NSL])
                        nc.tensor.matmul(ps2[:], ones[:], sq[:],
                                         start=(k == 0), stop=(k == KT - 1))
                    nc.vector.tensor_scalar_mul(msq[:, n0:n0 + NSL], ps2[:], 1.0 / H)
                nc.sync.dma_start(statsd[0:1, :], mu[:])         # -mean
                nc.vector.tensor_mul(mu[:], mu[:], mu[:])        # mean^2
                nc.vector.tensor_sub(msq[:], msq[:], mu[:])      # var
                nc.vector.tensor_scalar_add(msq[:], msq[:], EPS)
                nc.scalar.activation(msq[:], msq[:],
                                     mybir.ActivationFunctionType.Sqrt)  # sd
                nc.sync.dma_start(statsd[1:2, :], msq[:])
                nc.vector.reciprocal(mu[:], msq[:])              # r = 1/sd
                nc.sync.dma_start(statsd[2:3, :], mu[:])

            # ---- LM head on this core's vocab shard ----
            VS = 500
            with tc.tile_pool(name="hd_c", bufs=1) as hcp, \
                 tc.tile_pool(name="hd_w", bufs=2) as hwp, \
                 tc.tile_pool(name="hd_o", bufs=4) as hop, \
                 tc.tile_pool(name="hd_ps", bufs=4, space="PSUM") as hpp:
                extk = hcp.tile([128, N], BF16, tag="extk")
                nc.vector.memset(extk[:], 0.0)
                statw = hcp.tile([2, N], FP32, tag="statw")
                nc.sync.dma_start(statw[:], statsd[0:2, :])
                nc.vector.tensor_copy(extk[0:2, :], statw[:])    # rows: -mean, sd
                k12sb = hcp.tile([128, VSH], BF16, tag="k12sb")
                nc.vector.memset(k12sb[:], 0.0)
                nc.sync.dma_start(k12sb[0:2, :], k12[:, :])
                PR = min(128, N)
                r_sb = hcp.tile([PR, N // PR], FP32, tag="r_sb")
                nc.sync.dma_start(r_sb[:], statsd[2, :].rearrange("(m p) -> p m", p=PR))
                for vb in range(VSH // VS):
                    v0 = vb * VS
                    wv = hwp.tile([128, KT * VS], BF16, tag="wv")
                    for k in range(KT):
                        nc.sync.dma_start(
                            wv[:, k * VS:(k + 1) * VS],
                            wvT[128 * k:128 * (k + 1), v0:v0 + VS])
                    for m in range(N // 128):
                        ps = hpp.tile([128, VS], FP32)
                        nc.tensor.matmul(ps[:], extk[:, 128 * m:128 * (m + 1)],
                                         k12sb[:, v0:v0 + VS],
                                         start=True, stop=False)
                        for k in range(KT):
                            nc.tensor.matmul(
                                ps[:], h1sb[k][:, 128 * m:128 * (m + 1)],
                                wv[:, k * VS:(k + 1) * VS],
                                start=False, stop=(k == KT - 1))
                        ot = hop.tile([128, VS], FP32, tag="ho")
                        nc.scalar.activation(ot[:], ps[:], Ident,
                                             scale=r_sb[:, m:m + 1])
                        nc.sync.dma_start(
                            logits[128 * m:128 * (m + 1), v0:v0 + VS], ot[:])

    nc.finalize()
    return nc


_NC_CACHE = {}


def _get_nc(**kw):
    key = tuple(sorted(kw.items()))
    if key not in _NC_CACHE:
        _NC_CACHE[key] = build_nc(**kw)
    return _NC_CACHE[key]


def prep_inputs(input_ids, word_emb, pos_emb, W_ih, W_hh, b_ih, b_hh, ln_w, ln_b):
    """Host-side marshalling -> per-core in_maps."""
    input_ids = np.asarray(input_ids)
    Bv, Tv = input_ids.shape
    N = Bv * Tv
    ids_n = np.ascontiguousarray(input_ids.T.reshape(-1).astype(np.int32))
    pidx = (np.arange(N) // Bv).astype(np.int32)
    word_emb = np.asarray(word_emb, dtype=np.float32)
    base = {
        "ids": ids_n,
        "pidx": pidx,
        "wemb": word_emb,
        "pos": np.ascontiguousarray(np.asarray(pos_emb, dtype=np.float32)[:Tv]),
        "wih0": _prep_w(np.asarray(W_ih)[0], NP_BF16),
        "whh0": _prep_w(np.asarray(W_hh)[0]),
        "wih1": _prep_w(np.asarray(W_ih)[1], NP_BF16),
        "whh1": _prep_w(np.asarray(W_hh)[1]),
        "bias0": _prep_b(np.asarray(b_ih)[0], np.asarray(b_hh)[0]),
        "bias1": _prep_b(np.asarray(b_ih)[1], np.asarray(b_hh)[1]),
        "ident": np.eye(128, dtype=NP_BF16),
    }
    in_maps = []
    for k in range(NC):
        m = dict(base)
        shard = word_emb[k * VSH:(k + 1) * VSH, :]
        m["wvT"] = np.ascontiguousarray(
            (np.asarray(ln_w, np.float32)[:, None] * shard.T).astype(NP_BF16))
        m["k12"] = np.ascontiguousarray(np.stack([
            shard @ np.asarray(ln_w, np.float32),
            shard @ np.asarray(ln_b, np.float32)]).astype(NP_BF16))
        in_maps.append(m)
    return in_maps


def run_on_hw(inputs, T, trace=False, **build_kw):
    nc = _get_nc(T=T, **build_kw)
    in_maps = prep_inputs(**inputs)
    return run_bass_kernel_spmd(nc, in_maps, list(range(NC)), trace=trace)


def kernel(input_ids, word_emb, pos_emb, W_ih, W_hh, b_ih, b_hh, ln_w, ln_b):
    input_ids = np.asarray(input_ids)
    Bv, Tv = input_ids.shape
    nc = _get_nc(T=Tv)
    in_maps = prep_inputs(input_ids, word_emb, pos_emb, W_ih, W_hh,
                          b_ih, b_hh, ln_w, ln_b)
    res = run_bass_kernel_spmd(nc, in_maps, list(range(NC)))
    parts = []
    for k in range(NC):
        r = res.results[k]["logits"].reshape(Tv, Bv, VSH).transpose(1, 0, 2)
        parts.append(r)
    return np.ascontiguousarray(np.concatenate(parts, axis=2))


# revision 12
# speedup vs baseline: 3.2522x; 3.2522x over previous
"""Trainium2 Bass kernel for nn_CustomRNNmodel: embed -> 2-layer LSTM -> LN -> tied LM head.

Strategy (8 NeuronCores, SPMD, no collectives):
  - LSTM recurrence replicated on every core (per-step collectives are too
    expensive); vocab-dim of the tied LM head sharded 8 ways per core.
  - Two passes over time (all of layer 0, then layer 1) so each layer's
    input projection is a big parallel GEMM (X = x @ W_ih^T + b), leaving
    only W_hh @ h in the sequential loop.
  - All big GEMMs (input projections, LM head, embed transpose) run in
    bf16 (1 cycle/row on PE vs 4 for fp32); the recurrence stays fp32.
  - Recurrence matmuls: weights stationary, 4-way column tiling (M=32
    tiles at tile_position (0,32j)), K=128 chunks. Gate rows are permuted
    host-side so gates land in PSUM as [128 partitions, 4*slot] tiles with
    f/i/g/o contiguous col-blocks; h/c state tiles [128, 4*k] feed the next
    step's rhs directly (no transposes anywhere in the loop).

Layout bookkeeping (the invariant everything relies on):
  token index n = B*t + b  (b minor)
  gate row rho (after host permutation) = 32*c + m, c = 4*s + j,
    s = 8*blk + sg, blk in [f,i,g,o], p = 32*j + m, hidden unit u = 128*sg + p
  - PSUM gate tile for block blk: [p, 4*sg + b]
  - h/c state tile: [p, 4*sg + b]  <->  unit u = 128*sg + p  (sg = kappa)
  - next-step rhs for contraction chunk k = h[:, 4k:4k+4]
  - X (input projection) DRAM: [32 (s), 128 (p), N]
  - HT (hidden states) DRAM: [1024 (u), N]  (bf16)
"""
import numpy as np
import ml_dtypes
from contextlib import ExitStack

import concourse.bass as bass
import concourse.tile as tile
from concourse import bacc, mybir
from concourse.bass_utils import run_bass_kernel_spmd

V, H, L, B, NC = 32000, 1024, 2, 4, 8
VSH = V // NC                      # 4000 vocab rows per core
FP32 = mybir.dt.float32
BF16 = mybir.dt.bfloat16
NP_BF16 = ml_dtypes.bfloat16
EPS = 1e-5

# gate block order in our layout; reference order is [i, f, g, o]
BLK_TO_ORIG = [1, 0, 2, 3]         # blk 0=f 1=i 2=g 3=o -> orig gate index


def gate_row_perm():
    """perm[rho] = original row index in [0,4H) for permuted row rho."""
    rho = np.arange(4 * H)
    m = rho & 31
    c = rho >> 5
    j = c & 3
    s = c >> 2
    blk = s >> 3
    sg = s & 7
    u = 128 * sg + 32 * j + m
    orig_gate = np.array(BLK_TO_ORIG)[blk]
    return orig_gate * H + u


def _prep_w(w, np_dtype=np.float32):
    """[4H, H] weight -> permuted-transposed [H, 4H] contiguous."""
    perm = gate_row_perm()
    return np.ascontiguousarray(np.asarray(w)[perm, :].T.astype(np_dtype))


def _prep_b(b_ih, b_hh):
    perm = gate_row_perm()
    return np.ascontiguousarray(
        (np.asarray(b_ih) + np.asarray(b_hh))[perm].astype(np.float32))


def build_nc(T=1024, unroll=2, debug_outputs=False, staggered=True):
    """Build the full single-core SPMD program (same on all 8 cores; only the
    wvT input differs per core)."""
    nc = bacc.Bacc("TRN2", target_bir_lowering=False, debug=False)
    N = B * T                       # tokens
    NSL = min(512, N)               # n-slice width for GEMMs
    NB = N // NSL                   # n-slices
    KT = H // 128                   # 8 contraction chunks
    Sig = mybir.ActivationFunctionType.Sigmoid
    Tanh = mybir.ActivationFunctionType.Tanh
    Ident = mybir.ActivationFunctionType.Identity

    # ---- inputs ----
    ids = nc.dram_tensor("ids", [N], mybir.dt.int32, kind="ExternalInput")
    pidx = nc.dram_tensor("pidx", [N], mybir.dt.int32, kind="ExternalInput")
    wemb = nc.dram_tensor("wemb", [V, H], FP32, kind="ExternalInput")
    pos = nc.dram_tensor("pos", [T, H], FP32, kind="ExternalInput")
    wih0 = nc.dram_tensor("wih0", [H, 4 * H], BF16, kind="ExternalInput")
    whh0 = nc.dram_tensor("whh0", [H, 4 * H], BF16, kind="ExternalInput")
    wih1 = nc.dram_tensor("wih1", [H, 4 * H], BF16, kind="ExternalInput")
    whh1 = nc.dram_tensor("whh1", [H, 4 * H], BF16, kind="ExternalInput")
    bias0 = nc.dram_tensor("bias0", [4 * H], FP32, kind="ExternalInput")
    bias1 = nc.dram_tensor("bias1", [4 * H], FP32, kind="ExternalInput")
    k12 = nc.dram_tensor("k12", [2, VSH], BF16, kind="ExternalInput")
    wvT = nc.dram_tensor("wvT", [H, VSH], BF16, kind="ExternalInput")
    ident = nc.dram_tensor("ident", [128, 128], BF16, kind="ExternalInput")

    # ---- outputs / scratch DRAM ----
    logits = nc.dram_tensor("logits", [N, VSH], FP32, kind="ExternalOutput")
    kind_dbg = "ExternalOutput" if debug_outputs else "Internal"
    ftT = nc.dram_tensor("ftT", [H, N], BF16, kind=kind_dbg)
    x0d = nc.dram_tensor("x0d", [32, 128, N], FP32, kind=kind_dbg)
    h0T = nc.dram_tensor("h0T", [H, N], BF16, kind=kind_dbg)
    x1d = nc.dram_tensor("x1d", [32, 128, N], FP32, kind=kind_dbg)
    h1T = nc.dram_tensor("h1T", [H, N], BF16, kind=kind_dbg)
    statsd = nc.dram_tensor("statsd", [3, N], FP32, kind=kind_dbg)

    with tile.TileContext(nc) as tc:
        # ================= embed + transpose =================
        with tc.tile_pool(name="emb", bufs=3) as ep, \
             tc.tile_pool(name="emb_ps", bufs=4, space="PSUM") as epp, \
             tc.tile_pool(name="emb_c", bufs=1) as ecp:
            ident_sb = ecp.tile([128, 128], BF16, tag="ident")
            nc.sync.dma_start(ident_sb[:], ident[:, :])
            for q in range(N // 128):
                idt = ep.tile([128, 1], mybir.dt.int32, tag="idt")
                nc.sync.dma_start(idt[:], ids[128 * q:128 * (q + 1)].rearrange("(p o) -> p o", o=1))
                pidt = ep.tile([128, 1], mybir.dt.int32, tag="pidt")
                nc.sync.dma_start(pidt[:], pidx[128 * q:128 * (q + 1)].rearrange("(p o) -> p o", o=1))
                tok = ep.tile([128, H], FP32, tag="tok")
                nc.gpsimd.indirect_dma_start(
                    out=tok[:], out_offset=None, in_=wemb[:, :],
                    in_offset=bass.IndirectOffsetOnAxis(ap=idt[:, :1], axis=0))
                pst = ep.tile([128, H], FP32, tag="pst")
                nc.gpsimd.indirect_dma_start(
                    out=pst[:], out_offset=None, in_=pos[:, :],
                    in_offset=bass.IndirectOffsetOnAxis(ap=pidt[:, :1], axis=0))
                ftb = ep.tile([128, H], BF16, tag="ftb")
                nc.vector.tensor_add(ftb[:], tok[:], pst[:])
                for k in range(KT):
                    tp = epp.tile([128, 128], BF16)
                    nc.tensor.transpose(tp[:], ftb[:, 128 * k:128 * (k + 1)], ident_sb[:])
                    ftc = ep.tile([128, 128], BF16, tag="ftc")
                    nc.any.tensor_copy(ftc[:], tp[:])
                    nc.sync.dma_start(
                        ftT[128 * k:128 * (k + 1), 128 * q:128 * (q + 1)], ftc[:])

        # ============ helper: input-projection GEMM (bf16) ============
        # X = srcT-chunks^T @ Wprep + bias  -> xout [32, 128, N] fp32
        def xgemm(wdram, srcT, xout, biasdram):
            with tc.tile_pool(name="xg_w", bufs=1) as wp, \
                 tc.tile_pool(name="xg_sm", bufs=1) as smp, \
                 tc.tile_pool(name="xg_rhs", bufs=2) as rp, \
                 tc.tile_pool(name="xg_out", bufs=4) as op, \
                 tc.tile_pool(name="xg_ps", bufs=4, space="PSUM") as pp:
                wsb = wp.tile([128, KT * 4 * H], BF16, tag="wsb")
                for k in range(KT):
                    nc.sync.dma_start(
                        wsb[:, k * 4 * H:(k + 1) * 4 * H],
                        wdram[128 * k:128 * (k + 1), :])
                biassb = smp.tile([128, 32], FP32, tag="bias")
                nc.sync.dma_start(biassb[:], biasdram.rearrange("(s p) -> p s", p=128))
                for nb in range(NB):
                    n0 = nb * NSL
                    rhs = rp.tile([128, KT * NSL], BF16, tag="rhs")
                    for k in range(KT):
                        nc.sync.dma_start(
                            rhs[:, k * NSL:(k + 1) * NSL],
                            srcT[128 * k:128 * (k + 1), n0:n0 + NSL])
                    for s in range(32):
                        ps = pp.tile([128, NSL], FP32)
                        for k in range(KT):
                            nc.tensor.matmul(
                                ps[:],
                                wsb[:, 4 * H * k + 128 * s:4 * H * k + 128 * (s + 1)],
                                rhs[:, k * NSL:(k + 1) * NSL],
                                start=(k == 0), stop=(k == KT - 1))
                        ot = op.tile([128, NSL], FP32, tag="xo")
                        nc.scalar.activation(ot[:], ps[:], Ident,
                                             bias=biassb[:, s:s + 1], scale=1.0)
                        nc.sync.dma_start(xout[s, :, n0:n0 + NSL], ot[:])

        # ============ helper: LSTM recurrence pass (fp32) ============
        def lstm_pass(wdram, xd, hTout):
            hTr = hTout.rearrange("(sg p) n -> p sg n", p=128)
            with tc.tile_pool(name="rec_w", bufs=1) as wp, \
                 tc.tile_pool(name="rec_st", bufs=1) as sp, \
                 tc.tile_pool(name="rec_sc", bufs=2 * unroll) as scp, \
                 tc.tile_pool(name="rec_ps", bufs=unroll, space="PSUM") as rpp:
                wsb = wp.tile([128, KT * 4 * H], BF16, tag="wsb")
                for k in range(KT):
                    nc.sync.dma_start(
                        wsb[:, k * 4 * H:(k + 1) * 4 * H],
                        wdram[128 * k:128 * (k + 1), :])
                hb = sp.tile([128, 32], BF16, tag="hbst")
                c = sp.tile([128, 32], FP32, tag="c")
                nc.vector.memset(hb[:], 0.0)
                nc.vector.memset(c[:], 0.0)
                assert T % unroll == 0
                with tc.For_i(0, B * T, B * unroll,
                              staggered_reset=staggered,
                              hint_engines=(mybir.EngineType.PE,
                                            mybir.EngineType.DVE,
                                            mybir.EngineType.Activation,
                                            mybir.EngineType.SP)) as i:
                    for uu in range(unroll):
                        col = bass.ds(i + B * uu, B)
                        xt = scp.tile([128, 128], FP32, tag="xt")
                        nc.sync.dma_start(
                            xt[:].rearrange("p (s b) -> p s b", b=4),
                            xd[:, :, col].rearrange("s p b -> p s b"))
                        psb = [rpp.tile([128, 32], FP32, tag=f"ps{bb}", name=f"psb{bb}")
                               for bb in range(4)]
                        # gate blocks in order f, i, g, o
                        for blk in range(4):
                            for sg in range(8):
                                s = 8 * blk + sg
                                for k in range(KT):
                                    for j in range(4):
                                        cidx = 4 * s + j
                                        nc.tensor.matmul(
                                            psb[blk][32 * j:32 * (j + 1), 4 * sg:4 * (sg + 1)],
                                            wsb[:, 4 * H * k + 32 * cidx:4 * H * k + 32 * (cidx + 1)],
                                            hb[:, 4 * k:4 * (k + 1)],
                                            start=(k == 0), stop=(k == KT - 1),
                                            tile_position=(0, 32 * j))
                        pre = [scp.tile([128, 32], FP32, tag=f"pre{bb}", name=f"pre{bb}")
                               for bb in range(4)]
                        for blk in range(4):
                            nc.vector.tensor_add(pre[blk][:], psb[blk][:],
                                                 xt[:, 32 * blk:32 * (blk + 1)])
                        F = scp.tile([128, 32], FP32, tag="F")
                        I = scp.tile([128, 32], FP32, tag="I")
                        G = scp.tile([128, 32], FP32, tag="G")
                        O = scp.tile([128, 32], FP32, tag="O")
                        nc.scalar.activation(F[:], pre[0][:], Sig)
                        t1 = scp.tile([128, 32], FP32, tag="t1")
                        nc.vector.tensor_mul(t1[:], F[:], c[:])
                        nc.scalar.activation(I[:], pre[1][:], Sig)
                        nc.scalar.activation(G[:], pre[2][:], Tanh)
                        t2 = scp.tile([128, 32], FP32, tag="t2")
                        nc.vector.tensor_mul(t2[:], I[:], G[:])
                        nc.vector.tensor_add(c[:], t1[:], t2[:])
                        TC = scp.tile([128, 32], FP32, tag="TC")
                        nc.scalar.activation(TC[:], c[:], Tanh)
                        nc.scalar.activation(O[:], pre[3][:], Sig)
                        nc.vector.tensor_mul(hb[:], O[:], TC[:])
                        nc.sync.dma_start(
                            hTr[:, :, col],
                            hb[:].rearrange("p (sg b) -> p sg b", b=4))

        with nc.named_scope("xg0"):
            xgemm(wih0, ftT, x0d, bias0)
        with nc.named_scope("rec0"):
            lstm_pass(whh0, x0d, h0T)
        with nc.named_scope("xg1"):
            xgemm(wih1, h0T, x1d, bias1)
        with nc.named_scope("rec1"):
            lstm_pass(whh1, x1d, h1T)

        # ========== layernorm folded into LM head (bf16) ==========
        # logits[n,v] = r[n] * ( sum_u x[u,n]*Wv'[u,v] + (-mu[n])*K1[v] + sd[n]*K2[v] )
        # with Wv' = lnw*WvT (host), K1 = Wv@lnw, K2 = Wv@lnb (host), r = 1/sd.
        with tc.tile_pool(name="ln_res", bufs=1) as lp:
            h1sb = [lp.tile([128, N], BF16, tag=f"h1_{k}", name=f"h1sb{k}") for k in range(KT)]
            for k in range(KT):
                nc.sync.dma_start(h1sb[k][:], h1T[128 * k:128 * (k + 1), :])
            with tc.tile_pool(name="ln_tmp", bufs=1) as ltp, \
                 tc.tile_pool(name="ln_sc", bufs=2) as lsp, \
                 tc.tile_pool(name="ln_ps", bufs=2, space="PSUM") as lpp:
                ones = ltp.tile([128, 1], BF16, tag="ones")
                nc.vector.memset(ones[:], 1.0)
                mu = ltp.tile([1, N], FP32, tag="mu")
                msq = ltp.tile([1, N], FP32, tag="msq")
                for nb in range(NB):
                    n0 = nb * NSL
                    ps = lpp.tile([1, NSL], FP32, tag="lnps")
                    for k in range(KT):
                        nc.tensor.matmul(ps[:], ones[:], h1sb[k][:, n0:n0 + NSL],
                                         start=(k == 0), stop=(k == KT - 1))
                    # mu slot first holds -mean = -colsum/H
                    nc.vector.tensor_scalar_mul(mu[:, n0:n0 + NSL], ps[:], -1.0 / H)
                    sq = lsp.tile([128, NSL], BF16, tag="sq")
                    ps2 = lpp.tile([1, NSL], FP32, tag="lnps2")
                    for k in range(KT):
                        nc.vector.tensor_mul(sq[:], h1sb[k][:, n0:n0 + NSL],
                                             h1sb[k][:, n0:n0 + NSL])
                        nc.tensor.matmul(ps2[:], ones[:], sq[:],
                                         start=(k == 0), stop=(k == KT - 1))
                    nc.vector.tensor_scalar_mul(msq[:, n0:n0 + NSL], ps2[:], 1.0 / H)
                nc.sync.dma_start(statsd[0:1, :], mu[:])         # -mean
                nc.vector.tensor_mul(mu[:], mu[:], mu[:])        # mean^2
                nc.vector.tensor_sub(msq[:], msq[:], mu[:])      # var
                nc.vector.tensor_scalar_add(msq[:], msq[:], EPS)
                nc.scalar.activation(msq[:], msq[:],
                                     mybir.ActivationFunctionType.Sqrt)  # sd
                nc.sync.dma_start(statsd[1:2, :], msq[:])
                nc.vector.reciprocal(mu[:], msq[:])              # r = 1/sd
                nc.sync.dma_start(statsd[2:3, :], mu[:])

            # ---- LM head on this core's vocab shard ----
            VS = 500
            with tc.tile_pool(name="hd_c", bufs=1) as hcp, \
                 tc.tile_pool(name="hd_w", bufs=2) as hwp, \
                 tc.tile_pool(name="hd_o", bufs=4) as hop, \
                 tc.tile_pool(name="hd_ps", bufs=4, space="PSUM") as hpp:
                extk = hcp.tile([128, N], BF16, tag="extk")
                nc.vector.memset(extk[:], 0.0)
                statw = hcp.tile([2, N], FP32, tag="statw")
                nc.sync.dma_start(statw[:], statsd[0:2, :])
                nc.vector.tensor_copy(extk[0:2, :], statw[:])    # rows: -mean, sd
                k12sb = hcp.tile([128, VSH], BF16, tag="k12sb")
                nc.vector.memset(k12sb[:], 0.0)
                nc.sync.dma_start(k12sb[0:2, :], k12[:, :])
                PR = min(128, N)
                r_sb = hcp.tile([PR, N // PR], FP32, tag="r_sb")
                nc.sync.dma_start(r_sb[:], statsd[2, :].rearrange("(m p) -> p m", p=PR))
                for vb in range(VSH // VS):
                    v0 = vb * VS
                    wv = hwp.tile([128, KT * VS], BF16, tag="wv")
                    for k in range(KT):
                        nc.sync.dma_start(
                            wv[:, k * VS:(k + 1) * VS],
                            wvT[128 * k:128 * (k + 1), v0:v0 + VS])
                    for m in range(N // 128):
                        ps = hpp.tile([128, VS], FP32)
                        nc.tensor.matmul(ps[:], extk[:, 128 * m:128 * (m + 1)],
                                         k12sb[:, v0:v0 + VS],
                                         start=True, stop=False)
                        for k in range(KT):
                            nc.tensor.matmul(
                                ps[:], h1sb[k][:, 128 * m:128 * (m + 1)],
                                wv[:, k * VS:(k + 1) * VS],
                                start=False, stop=(k == KT - 1))
                        ot = hop.tile([128, VS], FP32, tag="ho")
                        nc.scalar.activation(ot[:], ps[:], Ident,
                                             scale=r_sb[:, m:m + 1])
                        nc.sync.dma_start(
                            logits[128 * m:128 * (m + 1), v0:v0 + VS], ot[:])

    nc.finalize()
    return nc


_NC_CACHE = {}


def _get_nc(**kw):
    key = tuple(sorted(kw.items()))
    if key not in _NC_CACHE:
        _NC_CACHE[key] = build_nc(**kw)
    return _NC_CACHE[key]


def prep_inputs(input_ids, word_emb, pos_emb, W_ih, W_hh, b_ih, b_hh, ln_w, ln_b):
    """Host-side marshalling -> per-core in_maps."""
    input_ids = np.asarray(input_ids)
    Bv, Tv = input_ids.shape
    N = Bv * Tv
    ids_n = np.ascontiguousarray(input_ids.T.reshape(-1).astype(np.int32))
    pidx = (np.arange(N) // Bv).astype(np.int32)
    word_emb = np.asarray(word_emb, dtype=np.float32)
    base = {
        "ids": ids_n,
        "pidx": pidx,
        "wemb": word_emb,
        "pos": np.ascontiguousarray(np.asarray(pos_emb, dtype=np.float32)[:Tv]),
        "wih0": _prep_w(np.asarray(W_ih)[0], NP_BF16),
        "whh0": _prep_w(np.asarray(W_hh)[0], NP_BF16),
        "wih1": _prep_w(np.asarray(W_ih)[1], NP_BF16),
        "whh1": _prep_w(np.asarray(W_hh)[1], NP_BF16),
        "bias0": _prep_b(np.asarray(b_ih)[0], np.asarray(b_hh)[0]),
        "bias1": _prep_b(np.asarray(b_ih)[1], np.asarray(b_hh)[1]),
        "ident": np.eye(128, dtype=NP_BF16),
    }
    in_maps = []
    for k in range(NC):
        m = dict(base)
        shard = word_emb[k * VSH:(k + 1) * VSH, :]
        m["wvT"] = np.ascontiguousarray(
            (np.asarray(ln_w, np.float32)[:, None] * shard.T).astype(NP_BF16))
        m["k12"] = np.ascontiguousarray(np.stack([
            shard @ np.asarray(ln_w, np.float32),
            shard @ np.asarray(ln_b, np.float32)]).astype(NP_BF16))
        in_maps.append(m)
    return in_maps


def run_on_hw(inputs, T, trace=False, **build_kw):
    nc = _get_nc(T=T, **build_kw)
    in_maps = prep_inputs(**inputs)
    return run_bass_kernel_spmd(nc, in_maps, list(range(NC)), trace=trace)


def kernel(input_ids, word_emb, pos_emb, W_ih, W_hh, b_ih, b_hh, ln_w, ln_b):
    input_ids = np.asarray(input_ids)
    Bv, Tv = input_ids.shape
    nc = _get_nc(T=Tv)
    in_maps = prep_inputs(input_ids, word_emb, pos_emb, W_ih, W_hh,
                          b_ih, b_hh, ln_w, ln_b)
    res = run_bass_kernel_spmd(nc, in_maps, list(range(NC)))
    parts = []
    for k in range(NC):
        r = res.results[k]["logits"].reshape(Tv, Bv, VSH).transpose(1, 0, 2)
        parts.append(r)
    return np.ascontiguousarray(np.concatenate(parts, axis=2))


# revision 14
# speedup vs baseline: 9.4944x; 2.9194x over previous
"""Trainium2 Bass kernel for nn_CustomRNNmodel: embed -> 2-layer LSTM -> LN -> tied LM head.

Strategy (8 NeuronCores, SPMD, no collectives):
  - LSTM recurrence replicated on every core (per-step collectives are too
    expensive); vocab-dim of the tied LM head sharded 8 ways per core.
  - Two passes over time (all of layer 0, then layer 1) so each layer's
    input projection is a big parallel GEMM (X = x @ W_ih^T + b), leaving
    only W_hh @ h in the sequential loop.
  - Everything PE-heavy runs in bf16 (1 cycle/row vs 4 for fp32; fp32
    self-loading matmuls measured at ~214ns each vs ~33ns for bf16
    LDW+MM). The c state and all gate nonlinearities stay fp32; h is
    kept bf16 (it is only ever a matmul operand / DMA source).
  - Recurrence matmuls: weights stationary, full-width [128,128] bf16
    weight loads (FWL-eligible), M=128, N=4(batch), K=128 chunks
    accumulated in PSUM. Gate rows are permuted host-side so gates land
    in PSUM as [128 partitions, 4*slot] tiles with f/i/g/o contiguous
    col-blocks; the h state tile [128, 4*k] feeds the next step's rhs
    directly (no transposes anywhere in the loop).

Layout bookkeeping (the invariant everything relies on):
  token index n = B*t + b  (b minor)
  gate row rho (after host permutation) = 32*c + m, c = 4*s + j,
    s = 8*blk + sg, blk in [f,i,g,o], p = 32*j + m, hidden unit u = 128*sg + p
  - PSUM gate tile for block blk: [p, 4*sg + b]
  - h/c state tile: [p, 4*sg + b]  <->  unit u = 128*sg + p  (sg = kappa)
  - next-step rhs for contraction chunk k = h[:, 4k:4k+4]
  - X (input projection) DRAM: [32 (s), 128 (p), N]
  - HT (hidden states) DRAM: [1024 (u), N]  (bf16)
"""
import numpy as np
import ml_dtypes
from contextlib import ExitStack

import concourse.bass as bass
import concourse.tile as tile
from concourse import bacc, mybir
from concourse.bass_utils import run_bass_kernel_spmd

V, H, L, B, NC = 32000, 1024, 2, 4, 8
VSH = V // NC                      # 4000 vocab rows per core
FP32 = mybir.dt.float32
BF16 = mybir.dt.bfloat16
NP_BF16 = ml_dtypes.bfloat16
EPS = 1e-5

# gate block order in our layout; reference order is [i, f, g, o]
BLK_TO_ORIG = [1, 0, 2, 3]         # blk 0=f 1=i 2=g 3=o -> orig gate index


def gate_row_perm():
    """perm[rho] = original row index in [0,4H) for permuted row rho."""
    rho = np.arange(4 * H)
    m = rho & 31
    c = rho >> 5
    j = c & 3
    s = c >> 2
    blk = s >> 3
    sg = s & 7
    u = 128 * sg + 32 * j + m
    orig_gate = np.array(BLK_TO_ORIG)[blk]
    return orig_gate * H + u


def _prep_w(w, np_dtype=np.float32):
    """[4H, H] weight -> permuted-transposed [H, 4H] contiguous."""
    perm = gate_row_perm()
    return np.ascontiguousarray(np.asarray(w)[perm, :].T.astype(np_dtype))


def _prep_b(b_ih, b_hh):
    perm = gate_row_perm()
    return np.ascontiguousarray(
        (np.asarray(b_ih) + np.asarray(b_hh))[perm].astype(np.float32))


def build_nc(T=1024, unroll=2, debug_outputs=False, staggered=True):
    """Build the full single-core SPMD program (same on all 8 cores; only the
    wvT input differs per core)."""
    nc = bacc.Bacc("TRN2", target_bir_lowering=False, debug=False)
    N = B * T                       # tokens
    NSL = min(512, N)               # n-slice width for GEMMs
    NB = N // NSL                   # n-slices
    KT = H // 128                   # 8 contraction chunks
    Sig = mybir.ActivationFunctionType.Sigmoid
    Tanh = mybir.ActivationFunctionType.Tanh
    Ident = mybir.ActivationFunctionType.Identity

    # ---- inputs ----
    ids = nc.dram_tensor("ids", [N], mybir.dt.int32, kind="ExternalInput")
    pidx = nc.dram_tensor("pidx", [N], mybir.dt.int32, kind="ExternalInput")
    wemb = nc.dram_tensor("wemb", [V, H], FP32, kind="ExternalInput")
    pos = nc.dram_tensor("pos", [T, H], FP32, kind="ExternalInput")
    wih0 = nc.dram_tensor("wih0", [H, 4 * H], BF16, kind="ExternalInput")
    whh0 = nc.dram_tensor("whh0", [H, 4 * H], BF16, kind="ExternalInput")
    wih1 = nc.dram_tensor("wih1", [H, 4 * H], BF16, kind="ExternalInput")
    whh1 = nc.dram_tensor("whh1", [H, 4 * H], BF16, kind="ExternalInput")
    bias0 = nc.dram_tensor("bias0", [4 * H], FP32, kind="ExternalInput")
    bias1 = nc.dram_tensor("bias1", [4 * H], FP32, kind="ExternalInput")
    k12 = nc.dram_tensor("k12", [2, VSH], BF16, kind="ExternalInput")
    wvT = nc.dram_tensor("wvT", [H, VSH], BF16, kind="ExternalInput")
    ident = nc.dram_tensor("ident", [128, 128], BF16, kind="ExternalInput")

    # ---- outputs / scratch DRAM ----
    logits = nc.dram_tensor("logits", [N, VSH], FP32, kind="ExternalOutput")
    kind_dbg = "ExternalOutput" if debug_outputs else "Internal"
    ftT = nc.dram_tensor("ftT", [H, N], BF16, kind=kind_dbg)
    x0d = nc.dram_tensor("x0d", [32, 128, N], FP32, kind=kind_dbg)
    h0T = nc.dram_tensor("h0T", [H, N], BF16, kind=kind_dbg)
    x1d = nc.dram_tensor("x1d", [32, 128, N], FP32, kind=kind_dbg)
    h1T = nc.dram_tensor("h1T", [H, N], BF16, kind=kind_dbg)
    statsd = nc.dram_tensor("statsd", [3, N], FP32, kind=kind_dbg)

    with tile.TileContext(nc) as tc:
        # ================= embed + transpose =================
        with tc.tile_pool(name="emb", bufs=3) as ep, \
             tc.tile_pool(name="emb_ps", bufs=4, space="PSUM") as epp, \
             tc.tile_pool(name="emb_c", bufs=1) as ecp:
            ident_sb = ecp.tile([128, 128], BF16, tag="ident")
            nc.sync.dma_start(ident_sb[:], ident[:, :])
            for q in range(N // 128):
                idt = ep.tile([128, 1], mybir.dt.int32, tag="idt")
                nc.sync.dma_start(idt[:], ids[128 * q:128 * (q + 1)].rearrange("(p o) -> p o", o=1))
                pidt = ep.tile([128, 1], mybir.dt.int32, tag="pidt")
                nc.sync.dma_start(pidt[:], pidx[128 * q:128 * (q + 1)].rearrange("(p o) -> p o", o=1))
                tok = ep.tile([128, H], FP32, tag="tok")
                nc.gpsimd.indirect_dma_start(
                    out=tok[:], out_offset=None, in_=wemb[:, :],
                    in_offset=bass.IndirectOffsetOnAxis(ap=idt[:, :1], axis=0))
                pst = ep.tile([128, H], FP32, tag="pst")
                nc.gpsimd.indirect_dma_start(
                    out=pst[:], out_offset=None, in_=pos[:, :],
                    in_offset=bass.IndirectOffsetOnAxis(ap=pidt[:, :1], axis=0))
                ftb = ep.tile([128, H], BF16, tag="ftb")
                nc.vector.tensor_add(ftb[:], tok[:], pst[:])
                for k in range(KT):
                    tp = epp.tile([128, 128], BF16)
                    nc.tensor.transpose(tp[:], ftb[:, 128 * k:128 * (k + 1)], ident_sb[:])
                    ftc = ep.tile([128, 128], BF16, tag="ftc")
                    nc.any.tensor_copy(ftc[:], tp[:])
                    nc.sync.dma_start(
                        ftT[128 * k:128 * (k + 1), 128 * q:128 * (q + 1)], ftc[:])

        # ============ helper: input-projection GEMM (bf16) ============
        # X = srcT-chunks^T @ Wprep + bias  -> xout [32, 128, N] fp32
        def xgemm(wdram, srcT, xout, biasdram):
            with tc.tile_pool(name="xg_w", bufs=1) as wp, \
                 tc.tile_pool(name="xg_sm", bufs=1) as smp, \
                 tc.tile_pool(name="xg_rhs", bufs=2) as rp, \
                 tc.tile_pool(name="xg_out", bufs=4) as op, \
                 tc.tile_pool(name="xg_ps", bufs=4, space="PSUM") as pp:
                wsb = wp.tile([128, KT * 4 * H], BF16, tag="wsb")
                for k in range(KT):
                    nc.sync.dma_start(
                        wsb[:, k * 4 * H:(k + 1) * 4 * H],
                        wdram[128 * k:128 * (k + 1), :])
                biassb = smp.tile([128, 32], FP32, tag="bias")
                nc.sync.dma_start(biassb[:], biasdram.rearrange("(s p) -> p s", p=128))
                for nb in range(NB):
                    n0 = nb * NSL
                    rhs = rp.tile([128, KT * NSL], BF16, tag="rhs")
                    for k in range(KT):
                        nc.sync.dma_start(
                            rhs[:, k * NSL:(k + 1) * NSL],
                            srcT[128 * k:128 * (k + 1), n0:n0 + NSL])
                    for s in range(32):
                        ps = pp.tile([128, NSL], FP32)
                        for k in range(KT):
                            nc.tensor.matmul(
                                ps[:],
                                wsb[:, 4 * H * k + 128 * s:4 * H * k + 128 * (s + 1)],
                                rhs[:, k * NSL:(k + 1) * NSL],
                                start=(k == 0), stop=(k == KT - 1))
                        ot = op.tile([128, NSL], FP32, tag="xo")
                        nc.scalar.activation(ot[:], ps[:], Ident,
                                             bias=biassb[:, s:s + 1], scale=1.0)
                        nc.sync.dma_start(xout[s, :, n0:n0 + NSL], ot[:])

        # ============ helper: LSTM recurrence pass (fp32) ============
        def lstm_pass(wdram, xd, hTout):
            hTr = hTout.rearrange("(sg p) n -> p sg n", p=128)
            with tc.tile_pool(name="rec_w", bufs=1) as wp, \
                 tc.tile_pool(name="rec_st", bufs=1) as sp, \
                 tc.tile_pool(name="rec_sc", bufs=2 * unroll) as scp, \
                 tc.tile_pool(name="rec_ps", bufs=unroll, space="PSUM") as rpp:
                wsb = wp.tile([128, KT * 4 * H], BF16, tag="wsb")
                for k in range(KT):
                    nc.sync.dma_start(
                        wsb[:, k * 4 * H:(k + 1) * 4 * H],
                        wdram[128 * k:128 * (k + 1), :])
                hb = sp.tile([128, 32], BF16, tag="hbst")
                c = sp.tile([128, 32], FP32, tag="c")
                nc.vector.memset(hb[:], 0.0)
                nc.vector.memset(c[:], 0.0)
                assert T % unroll == 0
                with tc.For_i(0, B * T, B * unroll,
                              staggered_reset=staggered,
                              hint_engines=(mybir.EngineType.PE,
                                            mybir.EngineType.DVE,
                                            mybir.EngineType.Activation,
                                            mybir.EngineType.SP)) as i:
                    for uu in range(unroll):
                        col = bass.ds(i + B * uu, B)
                        xt = scp.tile([128, 128], FP32, tag="xt")
                        nc.sync.dma_start(
                            xt[:].rearrange("p (s b) -> p s b", b=4),
                            xd[:, :, col].rearrange("s p b -> p s b"))
                        psb = [rpp.tile([128, 32], FP32, tag=f"ps{bb}", name=f"psb{bb}")
                               for bb in range(4)]
                        # gate blocks in order f, i, g, o; one full-width
                        # 128-col weight load per (s, k) — FWL-eligible bf16,
                        # 4x fewer MMs than 32-col col-tiling (whose loads
                        # serialize anyway: row groups conflict)
                        for blk in range(4):
                            for sg in range(8):
                                s = 8 * blk + sg
                                for k in range(KT):
                                    nc.tensor.matmul(
                                        psb[blk][:, 4 * sg:4 * (sg + 1)],
                                        wsb[:, 4 * H * k + 128 * s:4 * H * k + 128 * (s + 1)],
                                        hb[:, 4 * k:4 * (k + 1)],
                                        start=(k == 0), stop=(k == KT - 1))
                        pre = [scp.tile([128, 32], FP32, tag=f"pre{bb}", name=f"pre{bb}")
                               for bb in range(4)]
                        for blk in range(4):
                            nc.vector.tensor_add(pre[blk][:], psb[blk][:],
                                                 xt[:, 32 * blk:32 * (blk + 1)])
                        F = scp.tile([128, 32], FP32, tag="F")
                        I = scp.tile([128, 32], FP32, tag="I")
                        G = scp.tile([128, 32], FP32, tag="G")
                        O = scp.tile([128, 32], FP32, tag="O")
                        nc.scalar.activation(F[:], pre[0][:], Sig)
                        t1 = scp.tile([128, 32], FP32, tag="t1")
                        nc.vector.tensor_mul(t1[:], F[:], c[:])
                        nc.scalar.activation(I[:], pre[1][:], Sig)
                        nc.scalar.activation(G[:], pre[2][:], Tanh)
                        t2 = scp.tile([128, 32], FP32, tag="t2")
                        nc.vector.tensor_mul(t2[:], I[:], G[:])
                        nc.vector.tensor_add(c[:], t1[:], t2[:])
                        TC = scp.tile([128, 32], FP32, tag="TC")
                        nc.scalar.activation(TC[:], c[:], Tanh)
                        nc.scalar.activation(O[:], pre[3][:], Sig)
                        nc.vector.tensor_mul(hb[:], O[:], TC[:])
                        nc.sync.dma_start(
                            hTr[:, :, col],
                            hb[:].rearrange("p (sg b) -> p sg b", b=4))

        with nc.named_scope("xg0"):
            xgemm(wih0, ftT, x0d, bias0)
        with nc.named_scope("rec0"):
            lstm_pass(whh0, x0d, h0T)
        with nc.named_scope("xg1"):
            xgemm(wih1, h0T, x1d, bias1)
        with nc.named_scope("rec1"):
            lstm_pass(whh1, x1d, h1T)

        # ========== layernorm folded into LM head (bf16) ==========
        # logits[n,v] = r[n] * ( sum_u x[u,n]*Wv'[u,v] + (-mu[n])*K1[v] + sd[n]*K2[v] )
        # with Wv' = lnw*WvT (host), K1 = Wv@lnw, K2 = Wv@lnb (host), r = 1/sd.
        with tc.tile_pool(name="ln_res", bufs=1) as lp:
            h1sb = [lp.tile([128, N], BF16, tag=f"h1_{k}", name=f"h1sb{k}") for k in range(KT)]
            for k in range(KT):
                nc.sync.dma_start(h1sb[k][:], h1T[128 * k:128 * (k + 1), :])
            with tc.tile_pool(name="ln_tmp", bufs=1) as ltp, \
                 tc.tile_pool(name="ln_sc", bufs=2) as lsp, \
                 tc.tile_pool(name="ln_ps", bufs=2, space="PSUM") as lpp:
                ones = ltp.tile([128, 1], BF16, tag="ones")
                nc.vector.memset(ones[:], 1.0)
                mu = ltp.tile([1, N], FP32, tag="mu")
                msq = ltp.tile([1, N], FP32, tag="msq")
                for nb in range(NB):
                    n0 = nb * NSL
                    ps = lpp.tile([1, NSL], FP32, tag="lnps")
                    for k in range(KT):
                        nc.tensor.matmul(ps[:], ones[:], h1sb[k][:, n0:n0 + NSL],
                                         start=(k == 0), stop=(k == KT - 1))
                    # mu slot first holds -mean = -colsum/H
                    nc.vector.tensor_scalar_mul(mu[:, n0:n0 + NSL], ps[:], -1.0 / H)
                    sq = lsp.tile([128, NSL], BF16, tag="sq")
                    ps2 = lpp.tile([1, NSL], FP32, tag="lnps2")
                    for k in range(KT):
                        nc.vector.tensor_mul(sq[:], h1sb[k][:, n0:n0 + NSL],
                                             h1sb[k][:, n0:n0 + NSL])
                        nc.tensor.matmul(ps2[:], ones[:], sq[:],
                                         start=(k == 0), stop=(k == KT - 1))
                    nc.vector.tensor_scalar_mul(msq[:, n0:n0 + NSL], ps2[:], 1.0 / H)
                nc.sync.dma_start(statsd[0:1, :], mu[:])         # -mean
                nc.vector.tensor_mul(mu[:], mu[:], mu[:])        # mean^2
                nc.vector.tensor_sub(msq[:], msq[:], mu[:])      # var
                nc.vector.tensor_scalar_add(msq[:], msq[:], EPS)
                nc.scalar.activation(msq[:], msq[:],
                                     mybir.ActivationFunctionType.Sqrt)  # sd
                nc.sync.dma_start(statsd[1:2, :], msq[:])
                nc.vector.reciprocal(mu[:], msq[:])              # r = 1/sd
                nc.sync.dma_start(statsd[2:3, :], mu[:])

            # ---- LM head on this core's vocab shard ----
            VS = 500
            with tc.tile_pool(name="hd_c", bufs=1) as hcp, \
                 tc.tile_pool(name="hd_w", bufs=2) as hwp, \
                 tc.tile_pool(name="hd_o", bufs=4) as hop, \
                 tc.tile_pool(name="hd_ps", bufs=4, space="PSUM") as hpp:
                extk = hcp.tile([128, N], BF16, tag="extk")
                nc.vector.memset(extk[:], 0.0)
                statw = hcp.tile([2, N], FP32, tag="statw")
                nc.sync.dma_start(statw[:], statsd[0:2, :])
                nc.vector.tensor_copy(extk[0:2, :], statw[:])    # rows: -mean, sd
                k12sb = hcp.tile([128, VSH], BF16, tag="k12sb")
                nc.vector.memset(k12sb[:], 0.0)
                nc.sync.dma_start(k12sb[0:2, :], k12[:, :])
                PR = min(128, N)
                r_sb = hcp.tile([PR, N // PR], FP32, tag="r_sb")
                nc.sync.dma_start(r_sb[:], statsd[2, :].rearrange("(m p) -> p m", p=PR))
                for vb in range(VSH // VS):
                    v0 = vb * VS
                    wv = hwp.tile([128, KT * VS], BF16, tag="wv")
                    for k in range(KT):
                        nc.sync.dma_start(
                            wv[:, k * VS:(k + 1) * VS],
                            wvT[128 * k:128 * (k + 1), v0:v0 + VS])
                    for m in range(N // 128):
                        ps = hpp.tile([128, VS], FP32)
                        nc.tensor.matmul(ps[:], extk[:, 128 * m:128 * (m + 1)],
                                         k12sb[:, v0:v0 + VS],
                                         start=True, stop=False)
                        for k in range(KT):
                            nc.tensor.matmul(
                                ps[:], h1sb[k][:, 128 * m:128 * (m + 1)],
                                wv[:, k * VS:(k + 1) * VS],
                                start=False, stop=(k == KT - 1))
                        ot = hop.tile([128, VS], FP32, tag="ho")
                        nc.scalar.activation(ot[:], ps[:], Ident,
                                             scale=r_sb[:, m:m + 1])
                        nc.sync.dma_start(
                            logits[128 * m:128 * (m + 1), v0:v0 + VS], ot[:])

    nc.finalize()
    return nc


_NC_CACHE = {}


def _get_nc(**kw):
    key = tuple(sorted(kw.items()))
    if key not in _NC_CACHE:
        _NC_CACHE[key] = build_nc(**kw)
    return _NC_CACHE[key]


def prep_inputs(input_ids, word_emb, pos_emb, W_ih, W_hh, b_ih, b_hh, ln_w, ln_b):
    """Host-side marshalling -> per-core in_maps."""
    input_ids = np.asarray(input_ids)
    Bv, Tv = input_ids.shape
    N = Bv * Tv
    ids_n = np.ascontiguousarray(input_ids.T.reshape(-1).astype(np.int32))
    pidx = (np.arange(N) // Bv).astype(np.int32)
    word_emb = np.asarray(word_emb, dtype=np.float32)
    base = {
        "ids": ids_n,
        "pidx": pidx,
        "wemb": word_emb,
        "pos": np.ascontiguousarray(np.asarray(pos_emb, dtype=np.float32)[:Tv]),
        "wih0": _prep_w(np.asarray(W_ih)[0], NP_BF16),
        "whh0": _prep_w(np.asarray(W_hh)[0], NP_BF16),
        "wih1": _prep_w(np.asarray(W_ih)[1], NP_BF16),
        "whh1": _prep_w(np.asarray(W_hh)[1], NP_BF16),
        "bias0": _prep_b(np.asarray(b_ih)[0], np.asarray(b_hh)[0]),
        "bias1": _prep_b(np.asarray(b_ih)[1], np.asarray(b_hh)[1]),
        "ident": np.eye(128, dtype=NP_BF16),
    }
    in_maps = []
    for k in range(NC):
        m = dict(base)
        shard = word_emb[k * VSH:(k + 1) * VSH, :]
        m["wvT"] = np.ascontiguousarray(
            (np.asarray(ln_w, np.float32)[:, None] * shard.T).astype(NP_BF16))
        m["k12"] = np.ascontiguousarray(np.stack([
            shard @ np.asarray(ln_w, np.float32),
            shard @ np.asarray(ln_b, np.float32)]).astype(NP_BF16))
        in_maps.append(m)
    return in_maps


def run_on_hw(inputs, T, trace=False, **build_kw):
    nc = _get_nc(T=T, **build_kw)
    in_maps = prep_inputs(**inputs)
    return run_bass_kernel_spmd(nc, in_maps, list(range(NC)), trace=trace)


def kernel(input_ids, word_emb, pos_emb, W_ih, W_hh, b_ih, b_hh, ln_w, ln_b):
    input_ids = np.asarray(input_ids)
    Bv, Tv = input_ids.shape
    nc = _get_nc(T=Tv)
    in_maps = prep_inputs(input_ids, word_emb, pos_emb, W_ih, W_hh,
                          b_ih, b_hh, ln_w, ln_b)
    res = run_bass_kernel_spmd(nc, in_maps, list(range(NC)))
    parts = []
    for k in range(NC):
        r = res.results[k]["logits"].reshape(Tv, Bv, VSH).transpose(1, 0, 2)
        parts.append(r)
    return np.ascontiguousarray(np.concatenate(parts, axis=2))
